# revision 6
# baseline (speedup 1.0000x reference)
"""Trainium2 Bass kernel for GNN mean-aggregation message passing.

reference semantics (numpy):
    messages = x[src]                        # [E, F] gather
    seg_sum  = scatter_add(messages, dst)    # [N, F]
    seg_cnt  = scatter_add(ones, dst)        # [N, 1]
    out      = seg_sum / max(seg_cnt, 1)

Distribution: edges are sorted by destination node on the host and dst-node
ranges are sharded across the 8 NeuronCores (6250 nodes each), so every core
owns a disjoint slice of the output and no inter-core collective is needed.

Per core, the node table is padded to 128 f32 columns (512B rows — the
dma_gather row-size granularity; cols 0:64 = x, col 64 = 1.0) and split into
two halves so row indices fit dma_gather's int16 index format. For each
128-node dst sub-range, two dma_gather instructions pull all its edge
messages (edge i -> partition i%128, chunk i//128). A one-hot matrix built
from (dst_rel == iota) on the vector engine turns the segment-sum into PSUM
matmul accumulation; the gathered ones-column yields per-node counts in the
same matmul. The activation engine evicts PSUM scaled by 1/max(count, 1).
"""

import sys

if "/opt/trn_rl_repo" not in sys.path:
    sys.path.insert(0, "/opt/trn_rl_repo")

import numpy as np

import concourse.tile as tile
from concourse import bacc, bass, mybir
from concourse.bass_utils import run_bass_kernel_spmd

P = 128
F = 64  # feature dim
TW = 128  # padded table width (elements)
N_CORES = 8


def _build_nc(
    nr: int,  # number of 128-node sub-ranges per core
    r_last: int,  # valid rows in the last sub-range (<= 128)
    b_lo: int,  # low-half blocks of 128 edges per sub-range
    b_hi: int,  # high-half blocks
    n_lo: int,  # rows in low table
    n_hi: int,  # rows in high table
    onehot_batch: int = 4,
    sbuf_bufs: int = 3,
    psum_bufs: int = 4,
    n_queues: int = 4,
):
    nc = bacc.Bacc(num_swdge_queues=n_queues)
    b = b_lo + b_hi
    n_out = (nr - 1) * P + r_last

    xlo_ext = nc.declare_dram_parameter("xlo", [n_lo, TW], mybir.dt.float32, isOutput=False)
    xhi_ext = nc.declare_dram_parameter("xhi", [n_hi, TW], mybir.dt.float32, isOutput=False)
    ilo_ext = nc.declare_dram_parameter("ilo", [nr, P, 8 * b_lo], mybir.dt.int16, isOutput=False)
    ihi_ext = nc.declare_dram_parameter("ihi", [nr, P, 8 * b_hi], mybir.dt.int16, isOutput=False)
    dst_ext = nc.declare_dram_parameter("dstf", [nr, P, b], mybir.dt.float32, isOutput=False)
    out_ext = nc.declare_dram_parameter("out", [n_out, F], mybir.dt.float32, isOutput=True)

    with tile.TileContext(nc) as tc:
        with (
            tc.tile_pool(name="const", bufs=1) as const_pool,
            tc.tile_pool(name="idx", bufs=sbuf_bufs) as idx_pool,
            tc.tile_pool(name="msg", bufs=sbuf_bufs) as msg_pool,
            tc.tile_pool(name="onehot", bufs=2 * onehot_batch) as oh_pool,
            tc.tile_pool(name="evict", bufs=2) as ev_pool,
            tc.tile_pool(name="psum", bufs=psum_bufs, space="PSUM") as psum_pool,
        ):
            iota_i = const_pool.tile([P, 1, P], mybir.dt.int32)
            nc.gpsimd.iota(iota_i[:], pattern=[[1, P]], base=0, channel_multiplier=0)
            iota_f = const_pool.tile([P, 1, P], mybir.dt.float32)
            nc.vector.tensor_copy(out=iota_f[:], in_=iota_i[:])

            for r in range(nr):
                rows = P if r < nr - 1 else r_last
                il_t = idx_pool.tile([P, 8 * b_lo], mybir.dt.int16)
                nc.sync.dma_start(out=il_t[:], in_=ilo_ext[r])
                ih_t = idx_pool.tile([P, 8 * b_hi], mybir.dt.int16)
                nc.sync.dma_start(out=ih_t[:], in_=ihi_ext[r])
                dst_t = idx_pool.tile([P, b], mybir.dt.float32)
                nc.sync.dma_start(out=dst_t[:], in_=dst_ext[r])

                msg_t = msg_pool.tile([P, b, TW], mybir.dt.float32)
                nc.gpsimd.dma_gather(
                    out_ap=msg_t[:, 0:b_lo, :],
                    in_ap=xlo_ext[:, :],
                    idxs_ap=il_t[:],
                    num_idxs=P * b_lo,
                    num_idxs_reg=P * b_lo,
                    elem_size=TW,
                    queue_num=r % n_queues,
                    single_packet=False,
                )
                nc.gpsimd.dma_gather(
                    out_ap=msg_t[:, b_lo:b, :],
                    in_ap=xhi_ext[:, :],
                    idxs_ap=ih_t[:],
                    num_idxs=P * b_hi,
                    num_idxs_reg=P * b_hi,
                    elem_size=TW,
                    queue_num=(r + 1) % n_queues,
                    single_packet=False,
                )

                psum_t = psum_pool.tile([P, F + 1], mybir.dt.float32)
                for j0 in range(0, b, onehot_batch):
                    nb = min(onehot_batch, b - j0)
                    oh_t = oh_pool.tile([P, onehot_batch, P], mybir.dt.float32)
                    nc.vector.tensor_tensor(
                        out=oh_t[:, :nb, :],
                        in0=dst_t[:, j0 : j0 + nb, None].to_broadcast([P, nb, P]),
                        in1=iota_f[:].to_broadcast([P, nb, P]),
                        op=mybir.AluOpType.is_equal,
                    )
                    for j in range(j0, j0 + nb):
                        nc.tensor.matmul(
                            out=psum_t[:],
                            lhsT=oh_t[:, j - j0, :],
                            rhs=msg_t[:, j, 0 : F + 1],
                            start=(j == 0),
                            stop=(j == b - 1),
                        )

                cnt_t = ev_pool.tile([P, 1], mybir.dt.float32)
                nc.vector.tensor_scalar_max(cnt_t[:], psum_t[:, F : F + 1], 1.0)
                rec_t = ev_pool.tile([P, 1], mybir.dt.float32)
                nc.vector.reciprocal(rec_t[:], cnt_t[:])
                out_t = ev_pool.tile([P, F], mybir.dt.float32)
                nc.scalar.activation(
                    out_t[:],
                    psum_t[:, 0:F],
                    func=mybir.ActivationFunctionType.Copy,
                    scale=rec_t[:],
                )
                nc.sync.dma_start(out=out_ext[r * P : r * P + rows], in_=out_t[:rows])
    nc.compile()
    return nc


def _pack_idx(idx: np.ndarray, n_blocks: int) -> np.ndarray:
    """dma_gather idx layout: [128, 8*n_blocks] int16, index i at partition
    i%16, slot i//16, replicated across the 8 groups of 16 partitions."""
    w = 8 * n_blocks
    out16 = np.zeros((16, w), dtype=np.int16)
    if len(idx):
        i = np.arange(len(idx))
        out16[i % 16, i // 16] = idx.astype(np.int16)
    return np.tile(out16, (8, 1))


def _shard_inputs(x: np.ndarray, edge_idx: np.ndarray):
    n_nodes = x.shape[0]
    split = (n_nodes + 1) // 2
    src = np.ascontiguousarray(edge_idx[0]).astype(np.int64)
    dst = np.ascontiguousarray(edge_idx[1]).astype(np.int64)
    npc = n_nodes // N_CORES
    assert n_nodes % N_CORES == 0
    nr = (npc + P - 1) // P
    r_last = npc - (nr - 1) * P

    order = np.argsort(dst, kind="stable")
    src_s = src[order]
    dst_s = dst[order]

    core_of = dst_s // npc
    rel = dst_s - core_of * npc
    sub_of = rel // P
    flat = core_of * nr + sub_of
    n_ranges = N_CORES * nr
    is_lo = src_s < split
    cnt_lo = np.bincount(flat[is_lo], minlength=n_ranges)
    cnt_hi = np.bincount(flat[~is_lo], minlength=n_ranges)
    b_lo = max(1, int(np.max((cnt_lo + P - 1) // P)))
    b_hi = max(1, int(np.max((cnt_hi + P - 1) // P)))
    b = b_lo + b_hi

    starts = np.zeros(n_ranges + 1, dtype=np.int64)
    np.cumsum(np.bincount(flat, minlength=n_ranges), out=starts[1:])

    xx = np.zeros((n_nodes, TW), dtype=np.float32)
    xx[:, :F] = x
    xx[:, F] = 1.0

    in_maps = []
    for c in range(N_CORES):
        ilo = np.zeros((nr, P, 8 * b_lo), dtype=np.int16)
        ihi = np.zeros((nr, P, 8 * b_hi), dtype=np.int16)
        dstf = np.full((nr, P, b), -1.0, dtype=np.float32)
        for r in range(nr):
            s0, s1 = starts[c * nr + r], starts[c * nr + r + 1]
            sl = src_s[s0:s1]
            dl = (rel[s0:s1] - r * P).astype(np.float32)
            lo_m = sl < split
            for idx_h, d_h, blocks, base_blk, arr in (
                (sl[lo_m], dl[lo_m], b_lo, 0, ilo),
                (sl[~lo_m] - split, dl[~lo_m], b_hi, b_lo, ihi),
            ):
                n = len(idx_h)
                pad = np.zeros(blocks * P, dtype=np.int64)
                pad[:n] = idx_h
                arr[r] = _pack_idx(pad, blocks)
                if n:
                    i = np.arange(n)
                    dstf[r, i % P, base_blk + i // P] = d_h
        in_maps.append(
            {"xlo": xx[:split], "xhi": xx[split:], "ilo": ilo, "ihi": ihi, "dstf": dstf}
        )

    meta = dict(
        nr=nr, r_last=r_last, b_lo=b_lo, b_hi=b_hi,
        n_lo=split, n_hi=n_nodes - split, npc=npc,
    )
    return in_maps, meta


def run(x, edge_idx, trace: bool = False):
    """Returns (out [N, F] float32, exec_time_ns | None)."""
    x = np.asarray(x)
    edge_idx = np.asarray(edge_idx)
    in_maps, meta = _shard_inputs(x, edge_idx)
    nc = _build_nc(
        meta["nr"], meta["r_last"], meta["b_lo"], meta["b_hi"],
        meta["n_lo"], meta["n_hi"],
    )
    res = run_bass_kernel_spmd(
        nc, in_maps, core_ids=list(range(N_CORES)), trace=trace
    )
    out = np.concatenate([r["out"] for r in res.results], axis=0)
    return out.astype(np.float32), res.exec_time_ns


def kernel(x, edge_idx):
    out, _ = run(x, edge_idx)
    return out
